# revision 34
# baseline (speedup 1.0000x reference)
"""Trainium2 Bass kernel for nn_HadamardClassifier (self-contained).

Math: out = -scale * l2norm_rows(x) @ H + bias, with H the [2048, 14951]
top-left slice of the 16384x16384 Sylvester Hadamard matrix,
H[i, j] = (-1)^popcount(i & j).

Since row index i < 2048 uses only 11 bits, H[i, j] == H2048[i, j & 2047]:
the output is a periodic tiling of y = xn' @ H2048 (7.3x FLOP reduction).
Further, H2048 = H16 (x) H128 (Kronecker split at bit 7):
    y[m, jh*128 + jl] = sum_il H128[il, jl] * u[m, ih -> jh, il]
    u[m, jh, il]      = sum_ih H16[ih, jh] * xn'[m, ih*128 + il]
The H16 stage is a 4-stage FWHT over ih on the vector engine (bf16, 2x
mode); the H128 stage is PE transposes of u panels + K=128 bf16 matmuls.
Fan-out adds y + per-block bias in ONE DVE op per half chunk using a
stride-0 repeated access pattern on y, producing bf16 staging that a
SWDGE (gpsimd) DMA casts to fp32 on the way to HBM (cast runs at line
rate, ~347 GB/s measured).

All heavy compute is bf16 (tolerance gate is 2e-2; bf16 lands ~1e-3).

Sharding: data-parallel over batch, 8 cores x 512 rows. No collectives.
"""

import numpy as np

BATCH = 4096
IN_DIM = 2048
OUT_DIM = 14951
PAD_OUT = 14952            # even tail for DVE 2x mode
EPS = 1e-12
N_CORES = 8
M_PER_CORE = BATCH // N_CORES          # 512
N_CHUNKS = M_PER_CORE // 128           # 4 m-chunks of 128 rows
N_FULL_BLOCKS = OUT_DIM // IN_DIM      # 7
TAIL_COLS = OUT_DIM - N_FULL_BLOCKS * IN_DIM  # 615


def _hadamard(n):
    """Sylvester Hadamard matrix H[i,j] = (-1)^popcount(i&j), float32."""
    i = np.arange(n, dtype=np.uint32)[:, None]
    j = np.arange(n, dtype=np.uint32)[None, :]
    v = i & j
    pc = np.zeros_like(v)
    for b in range(int(n).bit_length()):
        pc += (v >> b) & 1
    return (1.0 - 2.0 * (pc & 1)).astype(np.float32)


def _patch_tile_drain():
    """This walrus build accepts only ONE sync-wait per instruction, but
    Tile's kernel-tail drain attaches the whole global clock to a single
    Drain ('Too many sync wait commands').  Split the waits onto a chain of
    single-wait sequencer nops instead."""
    import concourse.mybir as mybir
    import concourse.tile as tile
    from concourse.vector_clock import ScopedClock

    if getattr(tile.TileContext, "_drain_split_patched", False):
        return

    def _drain_and_barrier(self, tick_clock, wait_clock):
        nc = self.nc
        probe = nc.sync.nop()
        wait_clock.add_sem_waits(
            probe.ins, ScopedClock({None: tick_clock.global_clock})
        )
        si = probe.ins.sync_info
        waits = list(si.on_wait) if si is not None and si.on_wait else []
        if len(waits) > 1:
            si.on_wait = waits[:1]
            for w in waits[1:]:
                n = nc.sync.nop()
                n.ins.sync_info = mybir.SyncInfo(on_wait=[w], on_update=[])
        nc.sync.drain()
        nc.all_engine_barrier()
        assert self.sems is not None
        popped = nc._tile_sem_poison_stack.pop()
        assert popped is self._sem_poison
        nc.clear_and_free_semaphores(list(self.sems.allocated().values()))
        nc.all_engine_barrier()

    tile.TileContext._drain_and_barrier = _drain_and_barrier
    tile.TileContext._drain_split_patched = True


def _split_multiwait_instructions(nc):
    """This walrus build rejects instructions with more than one sync-wait.
    Hoist extra waits onto same-engine nop instructions inserted just before
    the offending instruction (engine queues execute in order, so waiting on
    the nops first is equivalent)."""
    import concourse.mybir as mybir

    n_split = 0
    for blk in nc.m.functions[0].blocks:
        new_list = []
        for inst in blk.instructions:
            si = inst.sync_info
            waits = list(si.on_wait) if si is not None and si.on_wait else []
            if len(waits) > 1:
                for k, w in enumerate(waits[:-1]):
                    nop = mybir.InstNoOp(
                        name=f"{inst.name}-wsplit{k}", ins=[], outs=[])
                    nop.engine = inst.engine
                    nop.sync_info = mybir.SyncInfo(on_wait=[w], on_update=[])
                    new_list.append(nop)
                    n_split += 1
                si.on_wait = waits[-1:]
            new_list.append(inst)
        blk.instructions = new_list
    return n_split


def _build_program():
    import concourse.bass as bass
    import concourse.mybir as mybir
    import concourse.tile as tile

    _patch_tile_drain()
    f32 = mybir.dt.float32
    bf16 = mybir.dt.bfloat16
    nc = bass.Bass()

    x_d = nc.dram_tensor("x", [M_PER_CORE, IN_DIM], f32, kind="ExternalInput")
    h128_d = nc.dram_tensor("h128b", [128, 128], bf16, kind="ExternalInput")
    ident_d = nc.dram_tensor("identb", [128, 128], bf16, kind="ExternalInput")
    bias_d = nc.dram_tensor("biasb", [128, PAD_OUT], bf16, kind="ExternalInput")
    nscale_d = nc.dram_tensor("nscale", [128, 1], f32, kind="ExternalInput")
    out_d = nc.dram_tensor("out", [M_PER_CORE, OUT_DIM], f32, kind="ExternalOutput")

    from contextlib import ExitStack

    with tile.TileContext(nc) as tc, ExitStack() as ctx:
        singles = ctx.enter_context(tc.tile_pool(name="singles", bufs=1))
        xpool = ctx.enter_context(tc.tile_pool(name="xpool", bufs=4))
        scrpool = ctx.enter_context(tc.tile_pool(name="scr", bufs=2))
        xnpool = ctx.enter_context(tc.tile_pool(name="xn", bufs=2))
        uapool = ctx.enter_context(tc.tile_pool(name="ua", bufs=2))
        ubpool = ctx.enter_context(tc.tile_pool(name="ub", bufs=2))
        utpool = ctx.enter_context(tc.tile_pool(name="ut", bufs=2))
        ypool = ctx.enter_context(tc.tile_pool(name="yp", bufs=2))
        outpool = ctx.enter_context(tc.tile_pool(name="outp", bufs=2))
        tp_ps = ctx.enter_context(tc.tile_pool(name="tp_ps", bufs=2, space="PSUM"))
        z_ps = ctx.enter_context(tc.tile_pool(name="z_ps", bufs=2, space="PSUM"))

        # --- inputs: order the HWDGE queue by first-use time.  x0 gates all
        # of chunk 0; bias slices are needed only at the fan-out adds. ---
        x_tiles = [
            xpool.tile([128, IN_DIM], f32, name=f"x{c}", tag="x")
            for c in range(N_CHUNKS)
        ]
        # x0 in column halves so the first cast starts after 512 KB, not 1 MB
        nc.sync.dma_start(out=x_tiles[0][:, :1024], in_=x_d[0:128, :1024])
        nc.sync.dma_start(out=x_tiles[0][:, 1024:], in_=x_d[0:128, 1024:])
        h128_s = singles.tile([128, 128], bf16)
        nc.sync.dma_start(out=h128_s, in_=h128_d[:, :])
        ident_s = singles.tile([128, 128], bf16)
        nc.sync.dma_start(out=ident_s, in_=ident_d[:, :])
        nscale_s = singles.tile([128, 1], f32)
        nc.sync.dma_start(out=nscale_s, in_=nscale_d[:, :])
        bias_s = singles.tile([128, PAD_OUT], bf16)
        nc.sync.dma_start(
            out=bias_s[:, :4 * IN_DIM], in_=bias_d[:, :4 * IN_DIM])
        nc.sync.dma_start(out=x_tiles[1], in_=x_d[128:256, :])
        nc.sync.dma_start(
            out=bias_s[:, 4 * IN_DIM:PAD_OUT], in_=bias_d[:, 4 * IN_DIM:])
        nc.sync.dma_start(out=x_tiles[2], in_=x_d[256:384, :])
        nc.sync.dma_start(out=x_tiles[3], in_=x_d[384:512, :])
        eps_s = singles.tile([128, 1], f32)
        nc.vector.memset(eps_s, EPS)

        for c in range(N_CHUNKS):
            rows = slice(c * 128, (c + 1) * 128)
            x_c = x_tiles[c]

            # --- cast raw x to bf16 (independent of the norm: FWHT commutes
            # with per-row scaling, so rs is applied AFTER the FWHT) ---
            xn = xnpool.tile([128, IN_DIM], bf16)
            ua = uapool.tile([128, IN_DIM], bf16)
            ub = ubpool.tile([128, IN_DIM], bf16)

            def butterfly(cur, nxt, s, lo, hi):
                cw = 128 << s                      # contiguous run: t*128 cols
                cv = cur[:, lo:hi].rearrange("p (g two c) -> p g two c",
                                             two=2, c=cw)
                nv = nxt[:, lo:hi].rearrange("p (g two c) -> p g two c",
                                             two=2, c=cw)
                nc.vector.tensor_add(
                    out=nv[:, :, 0], in0=cv[:, :, 0], in1=cv[:, :, 1])
                nc.vector.tensor_tensor(
                    nv[:, :, 1], cv[:, :, 0], cv[:, :, 1],
                    mybir.AluOpType.subtract)

            # --- FWHT over ih (4 butterfly stages on DVE, bf16 2x mode) ---
            # column index i = ih*128 + il; stage s pairs ih bits at 1<<s.
            # Stages 0-2 act within 1024-col halves, so cast + first three
            # stages run per-half (overlaps ACT cast with DVE butterflies);
            # stage 3 spans halves.  The row-norm squares are interleaved
            # with the casts on ACT so the sqrt is never queued behind the
            # NEXT chunk's casts.
            sq = scrpool.tile([128, 1024], f32, tag="sq")
            ss0 = scrpool.tile([128, 1], f32, tag="ss0")
            ss1 = scrpool.tile([128, 1], f32, tag="ss1")
            rs = scrpool.tile([128, 1], f32, tag="rs")
            for h in range(2):
                lo, hi = h * 1024, (h + 1) * 1024
                nc.scalar.copy(out=xn[:, lo:hi], in_=x_c[:, lo:hi])
                nc.scalar.activation(
                    out=sq, in_=x_c[:, lo:hi],
                    func=mybir.ActivationFunctionType.Square,
                    accum_out=(ss0 if h == 0 else ss1))
                butterfly(xn, ua, 0, lo, hi)
                butterfly(ua, ub, 1, lo, hi)
                butterfly(ub, ua, 2, lo, hi)
            butterfly(ua, ub, 3, 0, IN_DIM)

            # rs = -scale / sqrt(ss + eps)
            nc.vector.tensor_add(out=rs, in0=ss0, in1=ss1)
            nc.scalar.activation(
                out=rs, in_=rs, func=mybir.ActivationFunctionType.Sqrt,
                bias=eps_s)
            nc.vector.reciprocal(out=rs, in_=rs)
            nc.vector.tensor_mul(out=rs, in0=rs, in1=nscale_s)

            # rs is folded into the y evacuation (free on ACT); u stays
            # unscaled through the transposes and matmuls.
            u = ub

            # --- PE transpose u panels: uT[il, jh, m] = u[m, jh*128+il] ---
            evac = nc.scalar.copy
            uT = utpool.tile([128, 16, 128], bf16)
            for g in range(4):
                tp = tp_ps.tile([128, 512], bf16, tag="tp")
                for hh in range(4):
                    h = 4 * g + hh
                    nc.tensor.transpose(
                        tp[:, hh * 128:(hh + 1) * 128],
                        u[:, h * 128:(h + 1) * 128],
                        ident_s)
                evac(out=uT[:, 4 * g:4 * g + 4, :], in_=tp)

            # --- H128 stage: y[m, jh*128+jl] = sum_il uT[il, jh, m] H128[il, jl]
            # The per-row norm scale rs rides along on the PSUM evacuation.
            y_bf = ypool.tile([128, IN_DIM], bf16)
            for g in range(4):
                zp = z_ps.tile([128, 512], f32, tag="zp")
                for hh in range(4):
                    nc.tensor.matmul(
                        zp[:, hh * 128:(hh + 1) * 128],
                        lhsT=uT[:, 4 * g + hh, :], rhs=h128_s,
                        start=True, stop=True)
                ys = y_bf[:, g * 512:(g + 1) * 512]
                if c == 0:
                    nc.vector.tensor_scalar_mul(out=ys, in0=zp, scalar1=rs)
                else:
                    nc.scalar.activation(
                        out=ys, in_=zp,
                        func=mybir.ActivationFunctionType.Copy, scale=rs)

            # --- fan-out: out[m, 2048*b + r] = y[m, r] + bias[2048*b + r] ---
            # three DVE ops per chunk (stride-0 repeat of y) + one big SWDGE
            # cast-DMA (smaller pieces measurably drop the SWDGE stream
            # below its ~425 GB/s cap).
            out_sb = outpool.tile([128, PAD_OUT], bf16)
            y_ap = y_bf[:, :]

            def y_rep(nrep):
                return bass.AP(
                    tensor=y_ap.tensor, offset=y_ap.offset,
                    ap=[list(y_ap.ap[0])] + [[0, nrep]] + [list(y_ap.ap[1])])

            def add_blocks(b0, b1):
                nc.vector.tensor_add(
                    out=out_sb[:, b0 * IN_DIM:b1 * IN_DIM].rearrange(
                        "p (b c) -> p b c", b=b1 - b0),
                    in0=y_rep(b1 - b0),
                    in1=bias_s[:, b0 * IN_DIM:b1 * IN_DIM].rearrange(
                        "p (b c) -> p b c", b=b1 - b0))

            add_blocks(0, 4)
            add_blocks(4, 7)
            # tail: pad col 14951 computed, not stored
            nc.vector.tensor_add(
                out=out_sb[:, 7 * IN_DIM:PAD_OUT],
                in0=y_bf[:, :PAD_OUT - 7 * IN_DIM],
                in1=bias_s[:, 7 * IN_DIM:PAD_OUT])
            nc.gpsimd.dma_start(
                out=out_d[rows, :], in_=out_sb[:, :OUT_DIM])

    _split_multiwait_instructions(nc)
    return nc


_PROGRAM = None


def _get_program():
    global _PROGRAM
    if _PROGRAM is None:
        _PROGRAM = _build_program()
    return _PROGRAM


def _run(inputs, trace=False, tmpdir=None):
    import ml_dtypes
    from concourse.bass_utils import run_bass_kernel_spmd

    x = np.ascontiguousarray(np.asarray(inputs["x"], dtype=np.float32))
    scale = np.asarray(inputs["scale"], dtype=np.float32)
    bias = np.ascontiguousarray(np.asarray(inputs["bias"], dtype=np.float32))
    assert x.shape == (BATCH, IN_DIM) and bias.shape == (OUT_DIM,)

    h128b = _hadamard(128).astype(ml_dtypes.bfloat16)
    identb = np.eye(128, dtype=np.float32).astype(ml_dtypes.bfloat16)
    nscale = np.full((128, 1), -float(scale.reshape(-1)[0]), dtype=np.float32)
    bias_pad = np.zeros((PAD_OUT,), dtype=np.float32)
    bias_pad[:OUT_DIM] = bias
    biasb = np.ascontiguousarray(
        np.broadcast_to(bias_pad.astype(ml_dtypes.bfloat16), (128, PAD_OUT)))

    shards = x.reshape(N_CORES, M_PER_CORE, IN_DIM)
    in_maps = [
        {
            "x": np.ascontiguousarray(shards[i]),
            "h128b": h128b,
            "identb": identb,
            "biasb": biasb,
            "nscale": nscale,
        }
        for i in range(N_CORES)
    ]
    nc = _get_program()
    res = run_bass_kernel_spmd(
        nc, in_maps, core_ids=list(range(N_CORES)), trace=trace, tmpdir=tmpdir
    )
    out = np.concatenate([r["out"] for r in res.results], axis=0)
    return out, res


def kernel(x, scale, bias):
    out, _ = _run({"x": x, "scale": scale, "bias": bias})
    return out
